# revision 11
# baseline (speedup 1.0000x reference)
"""Trainium2 Bass kernel for the Mamba keyword-spotting model.

Pure data parallel: batch B=8 sharded 1 example per NeuronCore (8 cores).

Per-core dataflow (one example), everything D-major ([channel partitions, t free]):
  hT   = projW.T @ x                                   (PE)        [256, 1000]
  xzT  = in_projW.T @ hT  -> xh_pre (512), z (512)     (PE)
  xh   = silu(conv4(xh_pre) + conv_b)                  (DVE chain + ACT silu)
  dbcT = x_projW.T @ xh   -> dtT[16], Bm[16], Cm[16]   (PE)        [48, 1000]
  delta= softplus(dt_projW.T @ dtT + b)                (PE + ACT)  [512, 1000]
  du   = delta * xh                                    (DVE)
  per s in 0..16:                    (scan state dim)
    bm/cm = broadcast of Bm[s,:] / Cm[s,:] to 128 partitions (PE rank-1 matmul)
    per m in 0..4:                   (d_inner tile)
      dA  = exp(A[d,s] * delta)                        (ACT, per-partition scale)
      dBu = du * bm                                    (DVE)
      h   = scan(dA, dBu): h[t] = dA[t]*h[t-1]+dBu[t]  (DVE tensor_tensor_scan)
      y[m] += h * cm                                   (DVE)
  yg     = (xh * D + y) * silu(z)                      (DVE + ACT)
  pooled = sum_t yg                                    (DVE reduce)
  out    = WcombT.T @ pooled + fc_b                    (PE)  where
           Wcomb = fc_w @ out_proj_w / T   (host-folded: out_proj+mean+fc)
"""

import numpy as np
from contextlib import ExitStack

import concourse.bass as bass
import concourse.bacc as bacc
import concourse.mybir as mybir
import concourse.tile as tile

# Model dims (hardcoded per contract)
B = 8
C_IN = 80
T = 1000
D_MODEL = 256
D_STATE = 16
D_CONV = 4
D_INNER = 512
DT_RANK = 16
N_LABELS = 35
NM = D_INNER // 128  # 4 d_inner tiles
F32 = mybir.dt.float32

N_CORES = 8


def _build():
    nc = bacc.Bacc()
    AF = mybir.ActivationFunctionType
    OP = mybir.AluOpType

    # ---- DRAM params (per-core views; host pre-layouts them) ----
    xb = nc.declare_dram_parameter("xb", [C_IN, T], F32, isOutput=False)
    projwT = nc.declare_dram_parameter("projwT", [C_IN, D_MODEL], F32, isOutput=False)
    projb = nc.declare_dram_parameter("projb", [128, 2], F32, isOutput=False)
    ipwT = nc.declare_dram_parameter("ipwT", [D_MODEL, 2 * D_INNER], F32, isOutput=False)
    convw = nc.declare_dram_parameter("convw", [128, NM * D_CONV], F32, isOutput=False)
    convb = nc.declare_dram_parameter("convb", [128, NM], F32, isOutput=False)
    xpwT = nc.declare_dram_parameter("xpwT", [D_INNER, DT_RANK + 2 * D_STATE], F32, isOutput=False)
    dtpwT = nc.declare_dram_parameter("dtpwT", [DT_RANK, D_INNER], F32, isOutput=False)
    dtpb = nc.declare_dram_parameter("dtpb", [128, NM], F32, isOutput=False)
    a_neg = nc.declare_dram_parameter("a_neg", [128, NM * D_STATE], F32, isOutput=False)
    dw = nc.declare_dram_parameter("dw", [128, NM], F32, isOutput=False)
    wcombT = nc.declare_dram_parameter("wcombT", [D_INNER, N_LABELS], F32, isOutput=False)
    fcb = nc.declare_dram_parameter("fcb", [N_LABELS, 1], F32, isOutput=False)
    out_d = nc.declare_dram_parameter("out", [N_LABELS, 1], F32, isOutput=True)

    with tile.TileContext(nc) as tc, ExitStack() as ctx:
        wp = ctx.enter_context(tc.tile_pool(name="weights", bufs=1))
        pp = ctx.enter_context(tc.tile_pool(name="persist", bufs=1))
        mmp = ctx.enter_context(tc.tile_pool(name="mmpsum", bufs=2, space="PSUM"))
        bcp = ctx.enter_context(tc.tile_pool(name="bcpsum", bufs=2, space="PSUM"))
        sp = ctx.enter_context(tc.tile_pool(name="scan", bufs=3))
        tp = ctx.enter_context(tc.tile_pool(name="tmp", bufs=2))

        dma = nc.sync.dma_start

        # ---- load weights ----
        def wtile(shape, src_ap, tag):
            t = wp.tile(shape, F32, tag=tag)
            dma(out=t[:, :], in_=src_ap)
            return t

        xb_sb = wtile([C_IN, T], xb[:, :], "xb")
        projwT_sb = wtile([C_IN, D_MODEL], projwT[:, :], "projwT")
        projb_sb = wtile([128, 2], projb[:, :], "projb")
        ipwT_sb = [wtile([128, 2 * D_INNER], ipwT[k * 128:(k + 1) * 128, :], f"ipwT{k}")
                   for k in range(2)]
        convw_sb = wtile([128, NM * D_CONV], convw[:, :], "convw")
        convb_sb = wtile([128, NM], convb[:, :], "convb")
        xpwT_sb = [wtile([128, 48], xpwT[k * 128:(k + 1) * 128, :], f"xpwT{k}")
                   for k in range(NM)]
        dtpwT_sb = wtile([DT_RANK, D_INNER], dtpwT[:, :], "dtpwT")
        dtpb_sb = wtile([128, NM], dtpb[:, :], "dtpb")
        a_sb = wtile([128, NM * D_STATE], a_neg[:, :], "a_neg")
        dw_sb = wtile([128, NM], dw[:, :], "dw")
        wcombT_sb = [wtile([128, N_LABELS], wcombT[k * 128:(k + 1) * 128, :], f"wcombT{k}")
                     for k in range(NM)]
        fcb_sb = wtile([N_LABELS, 1], fcb[:, :], "fcb")

        ones_sb = wp.tile([1, 128], F32, tag="ones", name="ones")
        nc.vector.memset(ones_sb[:, :], 1.0)

        # ---- S1: hT = projwT.T @ xb  (+ proj_b) ----
        hT_sb = [pp.tile([128, T], F32, tag=f"hT{m}", name=f"hT{m}") for m in range(2)]
        for m in range(2):
            for n in range(2):
                ps = mmp.tile([128, 500], F32, tag="mm", name="mm")
                nc.tensor.matmul(ps[:, :], projwT_sb[:, m * 128:(m + 1) * 128],
                                 xb_sb[:, n * 500:(n + 1) * 500], start=True, stop=True)
                nc.scalar.activation(hT_sb[m][:, n * 500:(n + 1) * 500], ps[:, :],
                                     AF.Identity, bias=projb_sb[:, m:m + 1])

        # ---- S2+S3: in_proj (xh half) + conv + silu ----
        xh_sb = [pp.tile([128, T], F32, tag=f"xh{m}", name=f"xh{m}") for m in range(NM)]
        sg_sb = [pp.tile([128, T], F32, tag=f"sg{m}", name=f"sg{m}") for m in range(NM)]
        for m in range(NM):
            xp = tp.tile([128, T + D_CONV - 1], F32, tag="xhpre", name="xhpre")
            nc.vector.memset(xp[:, 0:D_CONV - 1], 0.0)
            for n in range(2):
                ps = mmp.tile([128, 500], F32, tag="mm", name="mm")
                for k in range(2):
                    nc.tensor.matmul(ps[:, :], ipwT_sb[k][:, m * 128:(m + 1) * 128],
                                     hT_sb[k][:, n * 500:(n + 1) * 500],
                                     start=(k == 0), stop=(k == 1))
                nc.vector.tensor_copy(xp[:, 3 + n * 500:3 + (n + 1) * 500], ps[:, :])
            cv = tp.tile([128, T], F32, tag="cv", name="cv")
            nc.gpsimd.tensor_scalar_mul(cv[:, :], xp[:, 0:T],
                                        convw_sb[:, 4 * m:4 * m + 1])
            for j in range(1, D_CONV):
                nc.gpsimd.scalar_tensor_tensor(cv[:, :], xp[:, j:j + T],
                                               convw_sb[:, 4 * m + j:4 * m + j + 1],
                                               cv[:, :], op0=OP.mult, op1=OP.add)
            # silu(cv + convb): sigmoid on ACT, then (cv+convb)*sig fused on DVE
            sig = tp.tile([128, T], F32, tag="sig", name="sig")
            nc.scalar.activation(sig[:, :], cv[:, :], AF.Sigmoid,
                                 bias=convb_sb[:, m:m + 1])
            nc.vector.scalar_tensor_tensor(xh_sb[m][:, :], cv[:, :],
                                           convb_sb[:, m:m + 1], sig[:, :],
                                           op0=OP.add, op1=OP.mult)
        # z half -> silu(z) directly from PSUM
        for m in range(NM):
            for n in range(2):
                ps = mmp.tile([128, 500], F32, tag="mm", name="mm")
                for k in range(2):
                    nc.tensor.matmul(ps[:, :],
                                     ipwT_sb[k][:, D_INNER + m * 128:D_INNER + (m + 1) * 128],
                                     hT_sb[k][:, n * 500:(n + 1) * 500],
                                     start=(k == 0), stop=(k == 1))
                # silu(z) = z * sigmoid(z), z only ever lives in PSUM
                sigz = tp.tile([128, 500], F32, tag="sigz", name="sigz")
                nc.scalar.activation(sigz[:, :], ps[:, :], AF.Sigmoid)
                nc.vector.tensor_mul(sg_sb[m][:, n * 500:(n + 1) * 500],
                                     ps[:, :], sigz[:, :])

        # ---- S4: dbcT = xpwT.T @ xh  -> [48, 1000] (dt | B | C rows) ----
        dbc_sb = pp.tile([48, T], F32, tag="dbc", name="dbc")
        for n in range(2):
            ps = mmp.tile([48, 500], F32, tag="mmdbc", name="mmdbc", bufs=1)
            for k in range(NM):
                nc.tensor.matmul(ps[:, :], xpwT_sb[k][:, :],
                                 xh_sb[k][:, n * 500:(n + 1) * 500],
                                 start=(k == 0), stop=(k == NM - 1))
            nc.vector.tensor_copy(dbc_sb[:, n * 500:(n + 1) * 500], ps[:, :])

        # ---- S5: delta = softplus(dtpwT.T @ dtT + dtpb) ----
        delta_sb = [pp.tile([128, T], F32, tag=f"delta{m}", name=f"delta{m}") for m in range(NM)]
        for m in range(NM):
            for n in range(2):
                ps = mmp.tile([128, 500], F32, tag="mm", name="mm")
                nc.tensor.matmul(ps[:, :], dtpwT_sb[:, m * 128:(m + 1) * 128],
                                 dbc_sb[0:DT_RANK, n * 500:(n + 1) * 500],
                                 start=True, stop=True)
                # softplus(x+b) = ln(1 + exp(x+b)); inputs are <~0 so exp is safe
                et = tp.tile([128, 500], F32, tag="et", name="et", bufs=1)
                nc.scalar.activation(et[:, :], ps[:, :], AF.Exp,
                                     bias=dtpb_sb[:, m:m + 1])
                nc.scalar.activation(delta_sb[m][:, n * 500:(n + 1) * 500],
                                     et[:, :], AF.Ln, bias=1.0)

        # ---- S6: du = delta * xh ----
        du_sb = [pp.tile([128, T], F32, tag=f"du{m}", name=f"du{m}") for m in range(NM)]
        for m in range(NM):
            nc.gpsimd.tensor_mul(du_sb[m][:, :], delta_sb[m][:, :], xh_sb[m][:, :])

        # ---- S7: selective scan, s outer so Bm/Cm broadcasts are shared over m ----
        y_sb = [pp.tile([128, T], F32, tag=f"y{m}", name=f"y{m}") for m in range(NM)]
        for s in range(D_STATE):
            # PE rank-1 broadcast needs rhs at base partition 0: DMA-copy the
            # Bm/Cm rows (partitions 16+s / 32+s) down to partition-0 tiles.
            brow = tp.tile([1, T], F32, tag="brow", name="brow", bufs=2)
            nc.sync.dma_start(out=brow[0:1, :],
                              in_=dbc_sb[DT_RANK + s:DT_RANK + s + 1, :])
            crow = tp.tile([1, T], F32, tag="crow", name="crow", bufs=2)
            nc.sync.dma_start(out=crow[0:1, :],
                              in_=dbc_sb[DT_RANK + D_STATE + s:DT_RANK + D_STATE + s + 1, :])
            bms = []
            cms = []
            for hf in range(2):
                tf = slice(hf * 500, (hf + 1) * 500)
                bm = bcp.tile([128, 500], F32, tag="bm", name="bm")
                nc.tensor.matmul(bm[:, :], ones_sb[0:1, 0:128], brow[0:1, tf],
                                 start=True, stop=True)
                cm = bcp.tile([128, 500], F32, tag="cm", name="cm")
                nc.tensor.matmul(cm[:, :], ones_sb[0:1, 0:128], crow[0:1, tf],
                                 start=True, stop=True)
                bms.append(bm)
                cms.append(cm)
            for m in range(NM):
                col = D_STATE * m + s
                dA = sp.tile([128, T], F32, tag="dA", name="dA")
                nc.scalar.activation(dA[:, :], delta_sb[m][:, :], AF.Exp,
                                     scale=a_sb[:, col:col + 1])
                dBu = sp.tile([128, T], F32, tag="dBu", name="dBu")
                for hf in range(2):
                    tf = slice(hf * 500, (hf + 1) * 500)
                    nc.vector.tensor_mul(dBu[:, tf], du_sb[m][:, tf], bms[hf][:, :])
                h = sp.tile([128, T], F32, tag="h", name="h")
                nc.vector.tensor_tensor_scan(h[:, 0:500], dA[:, 0:500], dBu[:, 0:500],
                                             0.0, op0=OP.mult, op1=OP.add)
                nc.vector.tensor_tensor_scan(h[:, 500:T], dA[:, 500:T], dBu[:, 500:T],
                                             h[:, 499:500], op0=OP.mult, op1=OP.add)
                if s == 0:
                    for hf in range(2):
                        tf = slice(hf * 500, (hf + 1) * 500)
                        nc.vector.tensor_mul(y_sb[m][:, tf], h[:, tf], cms[hf][:, :])
                else:
                    yps = sp.tile([128, T], F32, tag="yps", name="yps", bufs=2)
                    for hf in range(2):
                        tf = slice(hf * 500, (hf + 1) * 500)
                        nc.vector.tensor_mul(yps[:, tf], h[:, tf], cms[hf][:, :])
                    nc.gpsimd.tensor_add(y_sb[m][:, :], y_sb[m][:, :], yps[:, :])

        # ---- S8: gate + pool:  pooled[m] = sum_t (xh*D + y) * silu(z) ----
        pooled_sb = pp.tile([128, NM], F32, tag="pooled", name="pooled")
        for m in range(NM):
            gt = tp.tile([128, T], F32, tag="gt", name="gt")
            nc.vector.scalar_tensor_tensor(gt[:, :], xh_sb[m][:, :],
                                           dw_sb[:, m:m + 1], y_sb[m][:, :],
                                           op0=OP.mult, op1=OP.add)
            nc.vector.tensor_mul(gt[:, :], gt[:, :], sg_sb[m][:, :])
            nc.vector.tensor_reduce(pooled_sb[:, m:m + 1], gt[:, :],
                                    axis=mybir.AxisListType.X, op=OP.add)

        # ---- S9: out = wcombT.T @ pooled + fc_b ----
        fin = mmp.tile([N_LABELS, 1], F32, tag="fin", name="fin", bufs=1)
        for m in range(NM):
            nc.tensor.matmul(fin[:, :], wcombT_sb[m][:, :], pooled_sb[:, m:m + 1],
                             start=(m == 0), stop=(m == NM - 1))
        out_sb = pp.tile([N_LABELS, 1], F32, tag="outsb", name="outsb")
        nc.scalar.activation(out_sb[:, :], fin[:, :], AF.Identity, bias=fcb_sb[:, 0:1])
        dma(out=out_d[:, :], in_=out_sb[:, :])

    nc.compile()
    return nc


_NC = None


def _get_nc():
    global _NC
    if _NC is None:
        _NC = _build()
    return _NC


def host_prep(inputs):
    """Pre-layout weights so every DRAM->SBUF DMA is a contiguous copy."""
    g = {k: np.asarray(v, dtype=np.float32) for k, v in inputs.items()}
    p = {}
    p["projwT"] = np.ascontiguousarray(g["proj_w"].T)
    p["projb"] = np.ascontiguousarray(g["proj_b"].reshape(2, 128).T)
    p["ipwT"] = np.ascontiguousarray(g["in_proj_w"].T)
    p["convw"] = np.ascontiguousarray(
        g["conv_w"].reshape(NM, 128, D_CONV).transpose(1, 0, 2).reshape(128, NM * D_CONV))
    p["convb"] = np.ascontiguousarray(g["conv_b"].reshape(NM, 128).T)
    p["xpwT"] = np.ascontiguousarray(g["x_proj_w"].T)
    p["dtpwT"] = np.ascontiguousarray(g["dt_proj_w"].T)
    p["dtpb"] = np.ascontiguousarray(g["dt_proj_b"].reshape(NM, 128).T)
    p["a_neg"] = np.ascontiguousarray(
        (-np.exp(g["A_log"])).reshape(NM, 128, D_STATE).transpose(1, 0, 2)
        .reshape(128, NM * D_STATE))
    p["dw"] = np.ascontiguousarray(g["D"].reshape(NM, 128).T)
    p["wcombT"] = np.ascontiguousarray(
        (g["out_proj_w"].T @ g["fc_w"].T) / np.float32(T))
    p["fcb"] = np.ascontiguousarray(g["fc_b"].reshape(N_LABELS, 1))
    return p, np.ascontiguousarray(g["x"])


def _run(inputs, trace=False):
    from concourse.bass_utils import run_bass_kernel_spmd
    nc = _get_nc()
    p, x = host_prep(inputs)
    in_maps = [dict(p, xb=np.ascontiguousarray(x[b])) for b in range(N_CORES)]
    res = run_bass_kernel_spmd(nc, in_maps, list(range(N_CORES)), trace=trace)
    out = np.stack([np.asarray(res.results[i]["out"]).reshape(N_LABELS)
                    for i in range(N_CORES)]).astype(np.float32)
    return out, res


def kernel(**inputs):
    out, _ = _run(inputs)
    return out
